# revision 18
# baseline (speedup 1.0000x reference)
"""Trainium2 Bass kernel for nn_DTHyperNet (soft decision tree hypernetwork).

Contract: kernel(**inputs) takes the FULL unsharded inputs (B=8192) as
numpy arrays and returns the FULL [8192, 100] float32 output. Internally
the batch is sharded 8 ways (pure data parallel, weights replicated) and
one Bass/Tile program is compiled and run SPMD on NeuronCores 0-7.

Math (eval mode):
  trunk:  h = relu(bn(x @ w_in + b_in))  [+ residual gelu blocks, which
          collapse to identity when bn2 weight/bias are zero - detected
          from the actual input values and skipped]
  heads:  fi/fs = h @ w_fi/w_fs  (15 nodes x 512 features)
          lnc   = h @ w_lc       (16 leaves x 100 classes)
  per node: sd = sigmoid(sum_f softmax(fi)*(x - fs))
  routing coeff[leaf] = prod_d (sd or 1-sd) along the tree path
  out = sum_l coeff_l * lnc_l

Implementation notes (v2, fp8 DoubleRow):
  - fi/fs/lnc head matmuls run in fp8(e4m3) with perf_mode=DoubleRow:
    both operands carry interleaved contraction pairs [128, 2, free], so a
    512-contraction needs 2 matmuls instead of 4 and streams 2 rows/cycle
    (HW-measured 216ns for a [128,2,512] MM - full 2x over f16).
  - hT is written directly in fp8 by the trunk's Relu activation; all head
    weights are host-cast to fp8 and resident in SBUF (loaded once).
  - Wfs is host-NEGATED; for "I-path" node pairs the (x - fs) subtraction
    is folded into the PE: a constant fp8 identity matrix accumulates +x
    into the fs PSUM group (xT8 stationary), so psum = x - fs directly and
    the DVE tdiff pass disappears. Applied to a tunable subset of pairs to
    balance PE (~122us base) against DVE (the bottleneck otherwise).
  - per node: ACT computes P=exp(fi_psum) (f16 out) with free den accum;
    DVE does one fused scalar_tensor_tensor (psum_or_tdiff * P) with num
    accumulated by the op (stt is always 1x on this HW - measured).
  - leaf logits: evacuated from PSUM by ACT (copy->f16), weighted by the
    routing coeff with a 2x-mode tensor_tensor (class-major layout, coeff
    broadcast on the middle dim), then reduced over the 16 leaves with a
    strided pairwise add-tree (2x mode; tensor_reduce measured 1x).
  - This walrus build rejects instructions with more than one semaphore
    wait, so a post-pass splits multi-wait instructions by hoisting
    excess waits onto same-engine NOPs.
"""
import os
import sys
import types
import numpy as np
import ml_dtypes
from contextlib import ExitStack


def _install_axon_ntff_hook():
    """Expose the axon NTFF profiling hook under antenv.axon_hooks so
    run_bass_kernel_spmd(trace=True) works in this container. Harmless
    no-op when the hook or .so is unavailable."""
    if 'antenv.axon_hooks' in sys.modules:
        return
    try:
        import antenv
    except ImportError:
        return
    hook = None
    try:
        from trn_agent_boot.trn_boot import _ntff_profile_via_ctypes
        hook = _ntff_profile_via_ctypes('/opt/axon/libaxon_pjrt.so')
    except Exception:
        hook = None
    mod = types.ModuleType('antenv.axon_hooks')
    mod._hook = hook
    mod.get_axon_ntff_profile_hook = lambda: mod._hook
    mod.set_axon_ntff_profile_hook = lambda h: setattr(mod, '_hook', h)
    antenv.axon_hooks = mod
    sys.modules['antenv.axon_hooks'] = mod


_install_axon_ntff_hook()

import concourse.bass as bass
import concourse.tile as tile
import concourse.mybir as mybir
import bass_rust as _br
from concourse import bass_utils


def fix_sync_waits(nc, max_waits=1):
    """Split instructions with >max_waits sem waits: excess waits move to
    preceding same-engine InstNoOp instructions (this walrus build rejects
    multi-wait instructions)."""
    n_split = 0
    uid = 0
    for f in nc.m.functions:
        for bb in f.blocks:
            newl = []
            dirty = False
            for inst in bb.instructions:
                si = inst.sync_info
                if si is not None:
                    waits = list(si.on_wait or [])
                    if len(waits) > max_waits:
                        n_split += 1
                        dirty = True
                        excess = waits[:-max_waits]
                        keep = waits[-max_waits:]
                        for i in range(0, len(excess), max_waits):
                            nop = mybir.InstNoOp(name=f"waitnop{uid}", ins=[], outs=[])
                            uid += 1
                            nop.engine = inst.engine
                            nop.sync_info = _br.SyncInfo(
                                on_wait=excess[i:i+max_waits], on_update=[])
                            newl.append(nop)
                        inst.sync_info = _br.SyncInfo(
                            on_wait=keep, on_update=list(si.on_update or []))
                newl.append(inst)
            if dirty:
                bb.instructions = newl
    return n_split


F = 512; H = 512; C = 100; D = 4
NODES = 15; LEAVES = 16; NBLOCKS = 2
BS = 1024          # per-core batch shard
NT = BS // 128     # b-tiles per core
KT = H // 128      # contraction chunks of 128
EPS = 1e-5

f32 = mybir.dt.float32
f16 = mybir.dt.float16
f8 = mybir.dt.float8e4
AF = mybir.ActivationFunctionType
ALU = mybir.AluOpType
DR = mybir.MatmulPerfMode.DoubleRow
X = mybir.AxisListType.X

# node pairs; I_SET members use the PE identity-matmul path for (x - fs)
PAIRS = [(0, 1), (2, 3), (4, 5), (14,), (6, 7), (8, 9), (10, 11), (12, 13)]
I_SET = {1, 5}


def build_nc(skip_blocks):
    nc = bass.Bass("TRN2", target_bir_lowering=False, debug=False, num_devices=1)
    d = {}
    def din(name, shape, dt):
        d[name] = nc.dram_tensor(name, shape, dt, kind="ExternalInput").ap()
    # all wide inputs are pre-arranged on the host into the exact SBUF
    # image ([128, KT*cols], partition-major) so every DMA line is fully
    # contiguous (2-12KB) - the naive (k p) c rearrange DMAs only moved
    # 512B per descriptor line and paced the whole warmup at ~200GB/s
    din("xT16", [128, KT * BS], f16)
    din("x16", [128, NT * F], f16)
    din("xT8", [128, KT * BS], f8)
    din("I8c", [128, 2 * 2 * 512], f8)     # [p, (kp j f)] identity for x-fold
    din("W0", [128, KT * H], f16)
    din("c0", [H, 1], f32)
    if not skip_blocks:
        for i in range(NBLOCKS):
            din(f"W1_{i}", [128, KT * H], f16); din(f"c1_{i}", [H, 1], f32)
            din(f"W2_{i}", [128, KT * H], f16); din(f"c2_{i}", [H, 1], f32)
    din("Wfifs", [128, NODES * 2 * KT * F], f8)   # [wfi_n | wfs_n] blocks
    din("Wlc", [128, KT * LEAVES * C], f16)       # class-major (c*LEAVES+l)
    y_ap = nc.dram_tensor("y", [BS, C], f32, kind="ExternalOutput").ap()

    with tile.TileContext(nc) as tc, ExitStack() as ctx:
        per = ctx.enter_context(tc.tile_pool(name="per", bufs=1))

        # ---------------- resident loads (contiguous SBUF images) -------
        def load_img(name, cols, dt, nsplit=2):
            wide = per.tile([128, cols], dt, name=name, tag=name)
            step = cols // nsplit
            for s in range(nsplit):
                nc.sync.dma_start(wide[:, s*step:(s+1)*step],
                                  d[name][:, s*step:(s+1)*step])
            return wide

        xT_w = load_img("xT16", KT * BS, f16, nsplit=4)
        W0_w = load_img("W0", KT * H, f16, nsplit=1)
        c0_t = []
        for m in range(KT):
            tl = per.tile([128, 1], f32, name=f"c0t{m}")
            nc.sync.dma_start(tl[:], d["c0"][m*128:(m+1)*128, :])
            c0_t.append(tl)

        x16_w = load_img("x16", NT * F, f16, nsplit=1)
        xT8_w = load_img("xT8", KT * BS, f8, nsplit=1)
        I8_w = per.tile([128, 2048], f8, name="I8w")
        nc.sync.dma_start(I8_w[:], d["I8c"])
        wf_w = per.tile([128, NODES * 2 * KT * F], f8, name="wfifs")
        BL = 2 * KT * F           # one node's [wfi | wfs] block
        for n in range(NODES):
            nc.sync.dma_start(wf_w[:, n*BL:n*BL + KT*F],
                              d["Wfifs"][:, n*BL:n*BL + KT*F])
            nc.sync.dma_start(wf_w[:, n*BL + KT*F:(n+1)*BL],
                              d["Wfifs"][:, n*BL + KT*F:(n+1)*BL])
        wfi_w = [wf_w[:, n*BL:n*BL + KT*F] for n in range(NODES)]
        wfs_w = [wf_w[:, n*BL + KT*F:(n+1)*BL] for n in range(NODES)]
        wlc_w = load_img("Wlc", KT * LEAVES * C, f16)

        hT8 = per.tile([128, KT * BS], f8, name="hT8")
        hT8v = hT8[:].rearrange("p (k c) -> p k c", k=KT)
        hT16 = per.tile([128, KT * BS], f16, name="hT16f")
        hT16v = hT16[:].rearrange("p (k c) -> p k c", k=KT)
        xTv = xT_w[:].rearrange("p (k c) -> p k c", k=KT)
        W0v = W0_w[:].rearrange("p (k c) -> p k c", k=KT)
        xT8v = xT8_w[:].rearrange("p (k c) -> p k c", k=KT)
        I8v = I8_w[:].rearrange("p (kp j f) -> p kp j f", kp=2, j=2)
        wlcv = wlc_w[:].rearrange("p (k c) -> p k c", k=KT)

        # ---------------- phase 1: trunk ----------------
        def dense_layerT(psp, in_v, W_v, c_t, func, out_ap_fn):
            # out[m][:, bc] = func( sum_k W[k][:,m].T @ in[k][:,bc] + c[m] )
            for bc in range(BS // 512):
                for m in range(KT):
                    ps = psp.tile([128, 512], f32, name="tps", tag="tps")
                    for k in range(KT):
                        nc.tensor.matmul(
                            ps[:], W_v[:, k, m*128:(m+1)*128],
                            in_v[:, k, bc*512:(bc+1)*512],
                            start=(k == 0), stop=(k == KT - 1))
                    nc.scalar.activation(out_ap_fn(m, bc), ps[:], func,
                                         bias=c_t[m][:], scale=1.0)

        with tc.tile_pool(name="tps", bufs=2, space="PSUM") as tpsp:
            if skip_blocks:
                dense_layerT(tpsp, xTv, W0v, c0_t, AF.Relu,
                             lambda m, bc: hT16v[:, m, bc*512:(bc+1)*512])
            else:
                with tc.tile_pool(name="blk", bufs=1) as blk:
                    dense_layerT(tpsp, xTv, W0v, c0_t, AF.Relu,
                                 lambda m, bc: hT16v[:, m, bc*512:(bc+1)*512])
                    o1 = blk.tile([128, KT * BS], f16, name="o1")
                    o2 = blk.tile([128, KT * BS], f16, name="o2")
                    o1v = o1[:].rearrange("p (k c) -> p k c", k=KT)
                    o2v = o2[:].rearrange("p (k c) -> p k c", k=KT)
                    for i in range(NBLOCKS):
                        W1_w = load_img(f"W1_{i}", KT * H, f16)
                        c1_t = []
                        for m in range(KT):
                            tl = blk.tile([128, 1], f32, name=f"c1t{i}{m}")
                            nc.sync.dma_start(tl[:], d[f"c1_{i}"][m*128:(m+1)*128, :])
                            c1_t.append(tl)
                        dense_layerT(tpsp, hT16v,
                                     W1_w[:].rearrange("p (k c) -> p k c", k=KT),
                                     c1_t, AF.Gelu,
                                     lambda m, bc: o1v[:, m, bc*512:(bc+1)*512])
                        W2_w = load_img(f"W2_{i}", KT * H, f16)
                        c2_t = []
                        for m in range(KT):
                            tl = blk.tile([128, 1], f32, name=f"c2t{i}{m}")
                            nc.sync.dma_start(tl[:], d[f"c2_{i}"][m*128:(m+1)*128, :])
                            c2_t.append(tl)
                        dense_layerT(tpsp, o1v,
                                     W2_w[:].rearrange("p (k c) -> p k c", k=KT),
                                     c2_t, AF.Gelu,
                                     lambda m, bc: o2v[:, m, bc*512:(bc+1)*512])
                        nc.vector.tensor_add(hT16[:], hT16[:], o2[:])
        with nc.allow_low_precision("fp8 hT for softmax heads only"):
            for bc in range(BS // 512):
                for m in range(KT):
                    nc.vector.tensor_copy(hT8v[:, m, bc*512:(bc+1)*512],
                                          hT16v[:, m, bc*512:(bc+1)*512])

        # ---------------- phase 2: per-tile state ----------------
        den_t = [per.tile([128, NODES], f32, name=f"den{t}") for t in range(NT)]
        num_t = [per.tile([128, NODES], f32, name=f"num{t}") for t in range(NT)]
        lsb_t = [per.tile([128, LEAVES * C], f16, name=f"lsb{t}")
                 for t in range(NT)]

        CC = 4 * C    # 400-col psum chunks (4 leaves' worth in class-major)
        NCH = (LEAVES * C) // CC

        node_ctx = ExitStack()
        fi_pool = node_ctx.enter_context(tc.tile_pool(name="fips", bufs=2, space="PSUM"))
        fs_pool = node_ctx.enter_context(tc.tile_pool(name="fsps", bufs=2, space="PSUM"))
        sb2 = ctx.enter_context(tc.tile_pool(name="sb2", bufs=4))
        small = ctx.enter_context(tc.tile_pool(name="smal", bufs=3))


        def emit_pair(t, pair, use_I):
            nj = len(pair)
            w = nj * 512
            fi2 = fi_pool.tile([128, 1024], f32, name="fi2", tag="fi2")
            fs2 = fs_pool.tile([128, 1024], f32, name="fs2", tag="fs2")
            hsl = lambda kp: hT8v[:, 2*kp:2*kp+2, t*128:(t+1)*128]
            # PE: kp-major so consecutive MMs share the stationary operand
            for kp in range(2):
                for j, n in enumerate(pair):
                    nc.tensor.matmul(fi2[:, j*512:(j+1)*512], hsl(kp),
                                     wfi_w[n].rearrange(
                                         "p (k c) -> p k c", k=KT)[:, 2*kp:2*kp+2, :],
                                     start=(kp == 0), stop=(kp == 1),
                                     perf_mode=DR, skip_group_check=True)
                for j, n in enumerate(pair):
                    nc.tensor.matmul(fs2[:, j*512:(j+1)*512], hsl(kp),
                                     wfs_w[n].rearrange(
                                         "p (k c) -> p k c", k=KT)[:, 2*kp:2*kp+2, :],
                                     start=(kp == 0),
                                     stop=(kp == 1 and not use_I),
                                     perf_mode=DR, skip_group_check=True)
                if use_I:
                    for j in range(nj):
                        nc.tensor.matmul(fs2[:, j*512:(j+1)*512],
                                         xT8v[:, 2*kp:2*kp+2, t*128:(t+1)*128],
                                         I8v[:, kp], start=False, stop=(kp == 1),
                                         perf_mode=DR, skip_group_check=True)
            # ACT: P = exp(fi), free den accumulation
            P2 = sb2.tile([128, 1024], f16, name="P2", tag="P2")
            for j, n in enumerate(pair):
                nc.scalar.activation(P2[:, j*512:(j+1)*512],
                                     fi2[:, j*512:(j+1)*512], AF.Exp,
                                     accum_out=den_t[t][:, n:n+1])
            if use_I:
                # psum already holds (x - fs); fused multiply+num-accumulate
                for j, n in enumerate(pair):
                    qs = sb2.tile([128, 512], f16, name="qs", tag="qs")
                    nc.vector.scalar_tensor_tensor(
                        qs[:], fs2[:, j*512:(j+1)*512], 1.0,
                        P2[:, j*512:(j+1)*512], op0=ALU.mult, op1=ALU.mult,
                        accum_out=num_t[t][:, n:n+1])
            else:
                # DVE tdiff: psum holds -fs; tdiff = psum + x (batched)
                td2 = sb2.tile([128, w], f16, name="td2", tag="td2")
                xb = x16_w[:, t*F:(t+1)*F].unsqueeze(1).broadcast_to(
                    [128, nj, 512])
                nc.vector.scalar_tensor_tensor(
                    td2[:].rearrange("p (j f) -> p j f", j=nj),
                    fs2[:, 0:w].rearrange("p (j f) -> p j f", j=nj),
                    1.0, xb, op0=ALU.mult, op1=ALU.add)
                for j, n in enumerate(pair):
                    qs = sb2.tile([128, 512], f16, name="qs", tag="qs")
                    nc.vector.scalar_tensor_tensor(
                        qs[:], P2[:, j*512:(j+1)*512], 1.0,
                        td2[:, j*512:(j+1)*512], op0=ALU.mult, op1=ALU.mult,
                        accum_out=num_t[t][:, n:n+1])

        def finalize_a(t):
            rden = small.tile([128, NODES], f32, name="rden", tag="rden")
            nc.vector.reciprocal(rden[:], den_t[t][:])
            ratio = small.tile([128, NODES], f32, name="ratio", tag="ratio")
            nc.vector.tensor_tensor(ratio[:], num_t[t][:], rden[:], op=ALU.mult)
            # sigmoid via Exp (stays on the loaded Exp table)
            er = small.tile([128, NODES], f32, name="er", tag="er")
            nc.scalar.activation(er[:], ratio[:], AF.Exp, scale=-1.0)
            er1 = small.tile([128, NODES], f32, name="er1", tag="er1")
            nc.vector.tensor_scalar(er1[:], er[:], 1.0, None, op0=ALU.add)
            sd = small.tile([128, NODES], f16, name="sd", tag="sd")
            with nc.allow_low_precision("sd in (0,1); f16 ample for routing"):
                nc.vector.reciprocal(sd[:], er1[:])
            nsd = small.tile([128, NODES], f16, name="nsd", tag="nsd")
            nc.vector.tensor_scalar(nsd[:], sd[:], -1.0, 1.0,
                                    op0=ALU.mult, op1=ALU.add)
            return sd, nsd

        coeff_t = [per.tile([128, LEAVES], f16, name=f"coef{t}")
                   for t in range(NT)]

        def finalize_r(t, sd, nsd):
            # tree routing: coeff[leaf] = prod_d (sd | 1-sd) down the path
            us = []
            off = 0
            for dlev in range(1, D + 1):
                w = 1 << (dlev - 1)
                u = small.tile([128, 2 * w], f16, name=f"u{dlev}", tag=f"u{dlev}")
                uv = u[:].rearrange("p (a two) -> p a two", two=2)
                nc.gpsimd.tensor_copy(uv[:, :, 0:1], sd[:, off:off+w].unsqueeze(2))
                nc.gpsimd.tensor_copy(uv[:, :, 1:2], nsd[:, off:off+w].unsqueeze(2))
                us.append(u)
                off += w
            coeff = coeff_t[t]
            cur = us[0]
            for dlev in range(2, D + 1):
                w = 1 << dlev
                out = coeff if dlev == D else small.tile(
                    [128, w], f16, name=f"c{dlev}", tag=f"c{dlev}")
                rep = cur[:].unsqueeze(2).broadcast_to([128, w // 2, 2])
                ov = out[:].rearrange("p (a two) -> p a two", two=2)
                iv = us[dlev - 1][:].rearrange("p (a two) -> p a two", two=2)
                nc.gpsimd.tensor_mul(ov, rep, iv)
                cur = out
        def leaf_sum(t):
            # weighted leaf sum: q3 = lsb * coeff (2x-mode tt, class-major),
            # then strided pairwise add-tree over the 16 leaf slots
            coeff = coeff_t[t]
            q3 = small.tile([128, LEAVES * C], f16, name="q3", tag="q3")
            q3v = q3[:].rearrange("p (c l) -> p c l", l=LEAVES)
            lv = lsb_t[t][:].rearrange("p (c l) -> p c l", l=LEAVES)
            cv = coeff[:].unsqueeze(1).broadcast_to([128, C, LEAVES])
            nc.vector.tensor_tensor(q3v, lv, cv, op=ALU.mult)
            s1 = small.tile([128, C * 8], f16, name="s1", tag="s1")
            s1v = s1[:].rearrange("p (c l) -> p c l", l=8)
            nc.vector.tensor_tensor(s1v, q3v[:, :, 0:8], q3v[:, :, 8:16],
                                    op=ALU.add)
            s2 = small.tile([128, C * 4], f16, name="s2", tag="s2")
            s2v = s2[:].rearrange("p (c l) -> p c l", l=4)
            nc.vector.tensor_tensor(s2v, s1v[:, :, 0:4], s1v[:, :, 4:8],
                                    op=ALU.add)
            s3 = small.tile([128, C * 2], f16, name="s3", tag="s3")
            s3v = s3[:].rearrange("p (c l) -> p c l", l=2)
            nc.vector.tensor_tensor(s3v, s2v[:, :, 0:2], s2v[:, :, 2:4],
                                    op=ALU.add)
            outt = small.tile([128, C], f32, name="outt", tag="outt")
            nc.vector.tensor_tensor(outt[:].unsqueeze(2), s3v[:, :, 0:1],
                                    s3v[:, :, 1:2], op=ALU.add)
            nc.sync.dma_start(y_ap[t*128:(t+1)*128, :], outt[:])

        # finalize for tile t is emitted interleaved into tile t+1's pair
        # stream so the in-order engine queues never head-of-line block on
        # the small cross-engine sigmoid/routing chain
        fin_state = {}
        for t in range(NT):
            for pi, pair in enumerate(PAIRS):
                emit_pair(t, pair, use_I=(pi in I_SET))
                if t > 0 and pi == 0:
                    fin_state[t-1] = finalize_a(t-1)
                if t > 0 and pi == 2:
                    finalize_r(t-1, *fin_state.pop(t-1))
        fin_state[NT-1] = finalize_a(NT-1)
        finalize_r(NT-1, *fin_state.pop(NT-1))
        node_ctx.close()   # release fi/fs PSUM banks for the tail

        # ---------------- tail: lnc head + weighted leaf sum ----------
        with tc.tile_pool(name="lps", bufs=6, space="PSUM") as lps_pool:
            for t in range(NT):
                for ci in range(NCH):
                    cols = slice(ci * CC, (ci + 1) * CC)
                    lps = lps_pool.tile([128, CC], f32, name="lp", tag="lp")
                    for k in range(KT):
                        nc.tensor.matmul(lps[:],
                                         hT16v[:, k, t*128:(t+1)*128],
                                         wlcv[:, k, cols],
                                         start=(k == 0), stop=(k == KT - 1))
                    nc.scalar.copy(lsb_t[t][:, cols], lps[:])
                leaf_sum(t)

    return nc


# revision 19
# speedup vs baseline: 1.0309x; 1.0309x over previous
"""Trainium2 Bass kernel for nn_DTHyperNet (soft decision tree hypernetwork).

Contract: kernel(**inputs) takes the FULL unsharded inputs (B=8192) as
numpy arrays and returns the FULL [8192, 100] float32 output. Internally
the batch is sharded 8 ways (pure data parallel, weights replicated) and
one Bass/Tile program is compiled and run SPMD on NeuronCores 0-7.

Math (eval mode):
  trunk:  h = relu(bn(x @ w_in + b_in))  [+ residual gelu blocks, which
          collapse to identity when bn2 weight/bias are zero - detected
          from the actual input values and skipped]
  heads:  fi/fs = h @ w_fi/w_fs  (15 nodes x 512 features)
          lnc   = h @ w_lc       (16 leaves x 100 classes)
  per node: sd = sigmoid(sum_f softmax(fi)*(x - fs))
  routing coeff[leaf] = prod_d (sd or 1-sd) along the tree path
  out = sum_l coeff_l * lnc_l

Implementation notes (v2, fp8 DoubleRow):
  - fi/fs/lnc head matmuls run in fp8(e4m3) with perf_mode=DoubleRow:
    both operands carry interleaved contraction pairs [128, 2, free], so a
    512-contraction needs 2 matmuls instead of 4 and streams 2 rows/cycle
    (HW-measured 216ns for a [128,2,512] MM - full 2x over f16).
  - hT is written directly in fp8 by the trunk's Relu activation; all head
    weights are host-cast to fp8 and resident in SBUF (loaded once).
  - Wfs is host-NEGATED; for "I-path" node pairs the (x - fs) subtraction
    is folded into the PE: a constant fp8 identity matrix accumulates +x
    into the fs PSUM group (xT8 stationary), so psum = x - fs directly and
    the DVE tdiff pass disappears. Applied to a tunable subset of pairs to
    balance PE (~122us base) against DVE (the bottleneck otherwise).
  - per node: ACT computes P=exp(fi_psum) (f16 out) with free den accum;
    DVE does one fused scalar_tensor_tensor (psum_or_tdiff * P) with num
    accumulated by the op (stt is always 1x on this HW - measured).
  - leaf logits: evacuated from PSUM by ACT (copy->f16), weighted by the
    routing coeff with a 2x-mode tensor_tensor (class-major layout, coeff
    broadcast on the middle dim), then reduced over the 16 leaves with a
    strided pairwise add-tree (2x mode; tensor_reduce measured 1x).
  - This walrus build rejects instructions with more than one semaphore
    wait, so a post-pass splits multi-wait instructions by hoisting
    excess waits onto same-engine NOPs.
"""
import os
import sys
import types
import numpy as np
import ml_dtypes
from contextlib import ExitStack


def _install_axon_ntff_hook():
    """Expose the axon NTFF profiling hook under antenv.axon_hooks so
    run_bass_kernel_spmd(trace=True) works in this container. Harmless
    no-op when the hook or .so is unavailable."""
    if 'antenv.axon_hooks' in sys.modules:
        return
    try:
        import antenv
    except ImportError:
        return
    hook = None
    try:
        from trn_agent_boot.trn_boot import _ntff_profile_via_ctypes
        hook = _ntff_profile_via_ctypes('/opt/axon/libaxon_pjrt.so')
    except Exception:
        hook = None
    mod = types.ModuleType('antenv.axon_hooks')
    mod._hook = hook
    mod.get_axon_ntff_profile_hook = lambda: mod._hook
    mod.set_axon_ntff_profile_hook = lambda h: setattr(mod, '_hook', h)
    antenv.axon_hooks = mod
    sys.modules['antenv.axon_hooks'] = mod


_install_axon_ntff_hook()

import concourse.bass as bass
import concourse.tile as tile
import concourse.mybir as mybir
import bass_rust as _br
from concourse import bass_utils


def fix_sync_waits(nc, max_waits=1):
    """Split instructions with >max_waits sem waits: excess waits move to
    preceding same-engine InstNoOp instructions (this walrus build rejects
    multi-wait instructions)."""
    n_split = 0
    uid = 0
    for f in nc.m.functions:
        for bb in f.blocks:
            newl = []
            dirty = False
            for inst in bb.instructions:
                si = inst.sync_info
                if si is not None:
                    waits = list(si.on_wait or [])
                    if len(waits) > max_waits:
                        n_split += 1
                        dirty = True
                        excess = waits[:-max_waits]
                        keep = waits[-max_waits:]
                        for i in range(0, len(excess), max_waits):
                            nop = mybir.InstNoOp(name=f"waitnop{uid}", ins=[], outs=[])
                            uid += 1
                            nop.engine = inst.engine
                            nop.sync_info = _br.SyncInfo(
                                on_wait=excess[i:i+max_waits], on_update=[])
                            newl.append(nop)
                        inst.sync_info = _br.SyncInfo(
                            on_wait=keep, on_update=list(si.on_update or []))
                newl.append(inst)
            if dirty:
                bb.instructions = newl
    return n_split


F = 512; H = 512; C = 100; D = 4
NODES = 15; LEAVES = 16; NBLOCKS = 2
BS = 1024          # per-core batch shard
NT = BS // 128     # b-tiles per core
KT = H // 128      # contraction chunks of 128
EPS = 1e-5

f32 = mybir.dt.float32
f16 = mybir.dt.float16
f8 = mybir.dt.float8e4
AF = mybir.ActivationFunctionType
ALU = mybir.AluOpType
DR = mybir.MatmulPerfMode.DoubleRow
X = mybir.AxisListType.X

# node pairs; I_SET members use the PE identity-matmul path for (x - fs)
PAIRS = [(0, 1), (2, 3), (4, 5), (14,), (6, 7), (8, 9), (10, 11), (12, 13)]
I_SET = {1, 5}


def build_nc(skip_blocks):
    nc = bass.Bass("TRN2", target_bir_lowering=False, debug=False, num_devices=1)
    d = {}
    def din(name, shape, dt):
        d[name] = nc.dram_tensor(name, shape, dt, kind="ExternalInput").ap()
    # all wide inputs are pre-arranged on the host into the exact SBUF
    # image ([128, KT*cols], partition-major) so every DMA line is fully
    # contiguous (2-12KB) - the naive (k p) c rearrange DMAs only moved
    # 512B per descriptor line and paced the whole warmup at ~200GB/s
    din("xT16", [128, KT * BS], f16)
    din("x16", [128, NT * F], f16)
    din("xT8", [128, KT * BS], f8)
    din("I8c", [128, 2 * 2 * 512], f8)     # [p, (kp j f)] identity for x-fold
    din("W0", [128, KT * H], f16)
    din("c0", [H, 1], f32)
    if not skip_blocks:
        for i in range(NBLOCKS):
            din(f"W1_{i}", [128, KT * H], f16); din(f"c1_{i}", [H, 1], f32)
            din(f"W2_{i}", [128, KT * H], f16); din(f"c2_{i}", [H, 1], f32)
    din("Wfifs", [128, NODES * 2 * KT * F], f8)   # [wfi_n | wfs_n] blocks
    din("Wlc", [128, KT * LEAVES * C], f16)       # class-major (c*LEAVES+l)
    y_ap = nc.dram_tensor("y", [BS, C], f32, kind="ExternalOutput").ap()

    with tile.TileContext(nc) as tc, ExitStack() as ctx:
        per = ctx.enter_context(tc.tile_pool(name="per", bufs=1))

        # ---------------- resident loads (contiguous SBUF images) -------
        def load_img(name, cols, dt, nsplit=2):
            wide = per.tile([128, cols], dt, name=name, tag=name)
            step = cols // nsplit
            for s in range(nsplit):
                nc.sync.dma_start(wide[:, s*step:(s+1)*step],
                                  d[name][:, s*step:(s+1)*step])
            return wide

        xT_w = load_img("xT16", KT * BS, f16, nsplit=4)
        W0_w = load_img("W0", KT * H, f16, nsplit=1)
        c0_t = []
        for m in range(KT):
            tl = per.tile([128, 1], f32, name=f"c0t{m}")
            nc.sync.dma_start(tl[:], d["c0"][m*128:(m+1)*128, :])
            c0_t.append(tl)

        x16_w = load_img("x16", NT * F, f16, nsplit=1)
        xT8_w = load_img("xT8", KT * BS, f8, nsplit=1)
        I8_w = per.tile([128, 2048], f8, name="I8w")
        nc.sync.dma_start(I8_w[:], d["I8c"])
        wf_w = per.tile([128, NODES * 2 * KT * F], f8, name="wfifs")
        BL = 2 * KT * F           # one node's [wfi | wfs] block
        for n in range(NODES):
            nc.sync.dma_start(wf_w[:, n*BL:n*BL + KT*F],
                              d["Wfifs"][:, n*BL:n*BL + KT*F])
            nc.sync.dma_start(wf_w[:, n*BL + KT*F:(n+1)*BL],
                              d["Wfifs"][:, n*BL + KT*F:(n+1)*BL])
        wfi_w = [wf_w[:, n*BL:n*BL + KT*F] for n in range(NODES)]
        wfs_w = [wf_w[:, n*BL + KT*F:(n+1)*BL] for n in range(NODES)]
        wlc_w = load_img("Wlc", KT * LEAVES * C, f16)

        hT8 = per.tile([128, KT * BS], f8, name="hT8")
        hT8v = hT8[:].rearrange("p (k c) -> p k c", k=KT)
        hT16 = per.tile([128, KT * BS], f16, name="hT16f")
        hT16v = hT16[:].rearrange("p (k c) -> p k c", k=KT)
        xTv = xT_w[:].rearrange("p (k c) -> p k c", k=KT)
        W0v = W0_w[:].rearrange("p (k c) -> p k c", k=KT)
        xT8v = xT8_w[:].rearrange("p (k c) -> p k c", k=KT)
        I8v = I8_w[:].rearrange("p (kp j f) -> p kp j f", kp=2, j=2)
        wlcv = wlc_w[:].rearrange("p (k c) -> p k c", k=KT)

        # ---------------- phase 1: trunk ----------------
        def dense_layerT(psp, in_v, W_v, c_t, func, out_ap_fn):
            # out[m][:, bc] = func( sum_k W[k][:,m].T @ in[k][:,bc] + c[m] )
            for bc in range(BS // 512):
                for m in range(KT):
                    ps = psp.tile([128, 512], f32, name="tps", tag="tps")
                    for k in range(KT):
                        nc.tensor.matmul(
                            ps[:], W_v[:, k, m*128:(m+1)*128],
                            in_v[:, k, bc*512:(bc+1)*512],
                            start=(k == 0), stop=(k == KT - 1))
                    nc.scalar.activation(out_ap_fn(m, bc), ps[:], func,
                                         bias=c_t[m][:], scale=1.0)

        with tc.tile_pool(name="tps", bufs=2, space="PSUM") as tpsp:
            if skip_blocks:
                dense_layerT(tpsp, xTv, W0v, c0_t, AF.Relu,
                             lambda m, bc: hT16v[:, m, bc*512:(bc+1)*512])
            else:
                with tc.tile_pool(name="blk", bufs=1) as blk:
                    dense_layerT(tpsp, xTv, W0v, c0_t, AF.Relu,
                                 lambda m, bc: hT16v[:, m, bc*512:(bc+1)*512])
                    o1 = blk.tile([128, KT * BS], f16, name="o1")
                    o2 = blk.tile([128, KT * BS], f16, name="o2")
                    o1v = o1[:].rearrange("p (k c) -> p k c", k=KT)
                    o2v = o2[:].rearrange("p (k c) -> p k c", k=KT)
                    for i in range(NBLOCKS):
                        W1_w = load_img(f"W1_{i}", KT * H, f16)
                        c1_t = []
                        for m in range(KT):
                            tl = blk.tile([128, 1], f32, name=f"c1t{i}{m}")
                            nc.sync.dma_start(tl[:], d[f"c1_{i}"][m*128:(m+1)*128, :])
                            c1_t.append(tl)
                        dense_layerT(tpsp, hT16v,
                                     W1_w[:].rearrange("p (k c) -> p k c", k=KT),
                                     c1_t, AF.Gelu,
                                     lambda m, bc: o1v[:, m, bc*512:(bc+1)*512])
                        W2_w = load_img(f"W2_{i}", KT * H, f16)
                        c2_t = []
                        for m in range(KT):
                            tl = blk.tile([128, 1], f32, name=f"c2t{i}{m}")
                            nc.sync.dma_start(tl[:], d[f"c2_{i}"][m*128:(m+1)*128, :])
                            c2_t.append(tl)
                        dense_layerT(tpsp, o1v,
                                     W2_w[:].rearrange("p (k c) -> p k c", k=KT),
                                     c2_t, AF.Gelu,
                                     lambda m, bc: o2v[:, m, bc*512:(bc+1)*512])
                        nc.vector.tensor_add(hT16[:], hT16[:], o2[:])
        with nc.allow_low_precision("fp8 hT for softmax heads only"):
            for bc in range(BS // 512):
                for m in range(KT):
                    nc.vector.tensor_copy(hT8v[:, m, bc*512:(bc+1)*512],
                                          hT16v[:, m, bc*512:(bc+1)*512])

        # ---------------- phase 2: per-tile state ----------------
        den_t = [per.tile([128, NODES], f32, name=f"den{t}") for t in range(NT)]
        num_t = [per.tile([128, NODES], f32, name=f"num{t}") for t in range(NT)]
        lsb_t = [per.tile([128, LEAVES * C], f16, name=f"lsb{t}")
                 for t in range(NT)]

        CC = 4 * C    # 400-col psum chunks (4 leaves' worth in class-major)
        NCH = (LEAVES * C) // CC

        node_ctx = ExitStack()
        fi_pool = node_ctx.enter_context(tc.tile_pool(name="fips", bufs=2, space="PSUM"))
        fs_pool = node_ctx.enter_context(tc.tile_pool(name="fsps", bufs=2, space="PSUM"))
        sb2 = ctx.enter_context(tc.tile_pool(name="sb2", bufs=3))
        small = ctx.enter_context(tc.tile_pool(name="smal", bufs=2))


        def emit_pair(t, pair, use_I):
            nj = len(pair)
            w = nj * 512
            fi2 = fi_pool.tile([128, 1024], f32, name="fi2", tag="fi2")
            fs2 = fs_pool.tile([128, 1024], f32, name="fs2", tag="fs2")
            hsl = lambda kp: hT8v[:, 2*kp:2*kp+2, t*128:(t+1)*128]
            # PE: kp-major so consecutive MMs share the stationary operand
            for kp in range(2):
                for j, n in enumerate(pair):
                    nc.tensor.matmul(fi2[:, j*512:(j+1)*512], hsl(kp),
                                     wfi_w[n].rearrange(
                                         "p (k c) -> p k c", k=KT)[:, 2*kp:2*kp+2, :],
                                     start=(kp == 0), stop=(kp == 1),
                                     perf_mode=DR, skip_group_check=True)
                for j, n in enumerate(pair):
                    nc.tensor.matmul(fs2[:, j*512:(j+1)*512], hsl(kp),
                                     wfs_w[n].rearrange(
                                         "p (k c) -> p k c", k=KT)[:, 2*kp:2*kp+2, :],
                                     start=(kp == 0),
                                     stop=(kp == 1 and not use_I),
                                     perf_mode=DR, skip_group_check=True)
                if use_I:
                    for j in range(nj):
                        nc.tensor.matmul(fs2[:, j*512:(j+1)*512],
                                         xT8v[:, 2*kp:2*kp+2, t*128:(t+1)*128],
                                         I8v[:, kp], start=False, stop=(kp == 1),
                                         perf_mode=DR, skip_group_check=True)
            # ACT: P = exp(fi), free den accumulation
            P2 = sb2.tile([128, 1024], f16, name="P2", tag="P2")
            for j, n in enumerate(pair):
                nc.scalar.activation(P2[:, j*512:(j+1)*512],
                                     fi2[:, j*512:(j+1)*512], AF.Exp,
                                     accum_out=den_t[t][:, n:n+1])
            if use_I:
                # psum already holds (x - fs); fused multiply+num-accumulate
                for j, n in enumerate(pair):
                    qs = sb2.tile([128, 512], f16, name="qs", tag="qs")
                    nc.vector.scalar_tensor_tensor(
                        qs[:], fs2[:, j*512:(j+1)*512], 1.0,
                        P2[:, j*512:(j+1)*512], op0=ALU.mult, op1=ALU.mult,
                        accum_out=num_t[t][:, n:n+1])
            else:
                # DVE tdiff: psum holds -fs; tdiff = psum + x (batched)
                td2 = sb2.tile([128, w], f16, name="td2", tag="td2")
                xb = x16_w[:, t*F:(t+1)*F].unsqueeze(1).broadcast_to(
                    [128, nj, 512])
                nc.vector.scalar_tensor_tensor(
                    td2[:].rearrange("p (j f) -> p j f", j=nj),
                    fs2[:, 0:w].rearrange("p (j f) -> p j f", j=nj),
                    1.0, xb, op0=ALU.mult, op1=ALU.add)
                for j, n in enumerate(pair):
                    qs = sb2.tile([128, 512], f16, name="qs", tag="qs")
                    nc.vector.scalar_tensor_tensor(
                        qs[:], P2[:, j*512:(j+1)*512], 1.0,
                        td2[:, j*512:(j+1)*512], op0=ALU.mult, op1=ALU.mult,
                        accum_out=num_t[t][:, n:n+1])

        def finalize_a(t):
            rden = small.tile([128, NODES], f32, name="rden", tag="rden")
            nc.vector.reciprocal(rden[:], den_t[t][:])
            ratio = small.tile([128, NODES], f32, name="ratio", tag="ratio")
            nc.vector.tensor_tensor(ratio[:], num_t[t][:], rden[:], op=ALU.mult)
            # sigmoid via Exp (stays on the loaded Exp table)
            er = small.tile([128, NODES], f32, name="er", tag="er")
            nc.scalar.activation(er[:], ratio[:], AF.Exp, scale=-1.0)
            er1 = small.tile([128, NODES], f32, name="er1", tag="er1")
            nc.vector.tensor_scalar(er1[:], er[:], 1.0, None, op0=ALU.add)
            sd = small.tile([128, NODES], f16, name="sd", tag="sd")
            with nc.allow_low_precision("sd in (0,1); f16 ample for routing"):
                nc.vector.reciprocal(sd[:], er1[:])
            nsd = small.tile([128, NODES], f16, name="nsd", tag="nsd")
            nc.vector.tensor_scalar(nsd[:], sd[:], -1.0, 1.0,
                                    op0=ALU.mult, op1=ALU.add)
            return sd, nsd

        coeff_t = [per.tile([128, LEAVES], f16, name=f"coef{t}")
                   for t in range(NT)]

        def finalize_r(t, sd, nsd):
            # tree routing: coeff[leaf] = prod_d (sd | 1-sd) down the path
            us = []
            off = 0
            for dlev in range(1, D + 1):
                w = 1 << (dlev - 1)
                u = small.tile([128, 2 * w], f16, name=f"u{dlev}", tag=f"u{dlev}")
                uv = u[:].rearrange("p (a two) -> p a two", two=2)
                nc.gpsimd.tensor_copy(uv[:, :, 0:1], sd[:, off:off+w].unsqueeze(2))
                nc.gpsimd.tensor_copy(uv[:, :, 1:2], nsd[:, off:off+w].unsqueeze(2))
                us.append(u)
                off += w
            coeff = coeff_t[t]
            cur = us[0]
            for dlev in range(2, D + 1):
                w = 1 << dlev
                out = coeff if dlev == D else small.tile(
                    [128, w], f16, name=f"c{dlev}", tag=f"c{dlev}")
                rep = cur[:].unsqueeze(2).broadcast_to([128, w // 2, 2])
                ov = out[:].rearrange("p (a two) -> p a two", two=2)
                iv = us[dlev - 1][:].rearrange("p (a two) -> p a two", two=2)
                nc.gpsimd.tensor_mul(ov, rep, iv)
                cur = out
        def leaf_sum(t):
            # weighted leaf sum: q3 = lsb * coeff (2x-mode tt, class-major),
            # then strided pairwise add-tree over the 16 leaf slots
            coeff = coeff_t[t]
            q3 = small.tile([128, LEAVES * C], f16, name="q3", tag="q3")
            q3v = q3[:].rearrange("p (c l) -> p c l", l=LEAVES)
            lv = lsb_t[t][:].rearrange("p (c l) -> p c l", l=LEAVES)
            cv = coeff[:].unsqueeze(1).broadcast_to([128, C, LEAVES])
            nc.vector.tensor_tensor(q3v, lv, cv, op=ALU.mult)
            s1 = small.tile([128, C * 8], f16, name="s1", tag="s1")
            s1v = s1[:].rearrange("p (c l) -> p c l", l=8)
            nc.vector.tensor_tensor(s1v, q3v[:, :, 0:8], q3v[:, :, 8:16],
                                    op=ALU.add)
            s2 = small.tile([128, C * 4], f16, name="s2", tag="s2")
            s2v = s2[:].rearrange("p (c l) -> p c l", l=4)
            nc.vector.tensor_tensor(s2v, s1v[:, :, 0:4], s1v[:, :, 4:8],
                                    op=ALU.add)
            s3 = small.tile([128, C * 2], f16, name="s3", tag="s3")
            s3v = s3[:].rearrange("p (c l) -> p c l", l=2)
            nc.vector.tensor_tensor(s3v, s2v[:, :, 0:2], s2v[:, :, 2:4],
                                    op=ALU.add)
            outt = small.tile([128, C], f32, name="outt", tag="outt")
            nc.vector.tensor_tensor(outt[:].unsqueeze(2), s3v[:, :, 0:1],
                                    s3v[:, :, 1:2], op=ALU.add)
            nc.sync.dma_start(y_ap[t*128:(t+1)*128, :], outt[:])

        # finalize for tile t is emitted interleaved into tile t+1's pair
        # stream so the in-order engine queues never head-of-line block on
        # the small cross-engine sigmoid/routing chain
        fin_state = {}
        for t in range(NT):
            for pi, pair in enumerate(PAIRS):
                emit_pair(t, pair, use_I=(pi in I_SET))
                if t > 0 and pi == 0:
                    fin_state[t-1] = finalize_a(t-1)
                if t > 0 and pi == 2:
                    finalize_r(t-1, *fin_state.pop(t-1))
        fin_state[NT-1] = finalize_a(NT-1)
        finalize_r(NT-1, *fin_state.pop(NT-1))
        node_ctx.close()   # release fi/fs PSUM banks for the tail

        # ---------------- tail: lnc head + weighted leaf sum ----------
        with tc.tile_pool(name="lps", bufs=6, space="PSUM") as lps_pool:
            for t in range(NT):
                for ci in range(NCH):
                    cols = slice(ci * CC, (ci + 1) * CC)
                    lps = lps_pool.tile([128, CC], f32, name="lp", tag="lp")
                    for k in range(KT):
                        nc.tensor.matmul(lps[:],
                                         hT16v[:, k, t*128:(t+1)*128],
                                         wlcv[:, k, cols],
                                         start=(k == 0), stop=(k == KT - 1))
                    nc.scalar.copy(lsb_t[t][:, cols], lps[:])
                leaf_sum(t)

    return nc
